# revision 1
# baseline (speedup 1.0000x reference)
"""Trainium2 Bass kernel for Swin-style windowed cosine attention.

Problem: nn_Attention_8100308321041
  q,k,v: [512, 8, 256, 16] f32; table: [961, 8]; index: [65536] i64;
  mask: [64, 256, 256] f32; out: [512, 256, 128] f32.

Strategy (8 NeuronCores, pure data-parallel):
  - Shard window-instances b by (b % 64) % 8 == core  -> 64 instances/core,
    ordered (wl, img) so each per-window bias+mask chunk is fetched once and
    reused across 8 images while the next chunk prefetches.
  - Host prep: l2-normalize q/k, transpose to a bf16 4-head row-group layout
    (partition 32*g + d) so four heads' K=16 QK matmuls occupy distinct PE
    row groups (tile_position) and FWL hides LDWEIGHTS; gather table[index]
    -> bias, combine bias+mask into C (bf16, ScalarE path) and C' = A*C + B
    (fp32, VectorE Schraudolph path), build v_aug with a ones column (fused
    softmax denominator).
  - Device per (window, head):
      S'[m,n] = kT.T @ qT     (bf16 matmul, 2 m-chunks, PSUM; C preloaded
                               via identity-stationary matmul on path A)
      path A (~5/8 of head-pairs): P' = exp(S'+C) on ScalarE (bf16,
        batched 2 heads per instruction)
      path B (rest): P' = bitcast_bf16(int16(A*S' + C')) on VectorE
        (Schraudolph exp with the bias-add fused into one instruction)
      out[n, 0:16|denom] = P'.T @ v_aug   (4 matmuls, K=128 chunks)
      out = out[:, :16] * (1/denom)       (VectorE, one op per window)
"""

import os
import sys

sys.path.insert(0, "/opt/trn_rl_repo")

import numpy as np
import ml_dtypes

import concourse.bass as bass
import concourse.bacc as bacc
import concourse.mybir as mybir
from concourse import tile
from concourse.bass_utils import run_bass_kernel_spmd

BF16 = ml_dtypes.bfloat16
FP8 = ml_dtypes.float8_e4m3

B_, H, N, D = 512, 8, 256, 16
NW = 64          # windows per image
M_CORES = 8
IMG = B_ // NW   # 8 images
WL = NW // M_CORES  # 8 distinct windows per core
NI = IMG * WL    # 64 instances per core
HD = H * D       # 128
EPS = 1e-12
NB_H = 6         # heads using the additive-C + ScalarE exp path
NP_H = H - 4     # heads 4..7 have the pre-scaled C' (fp32) available
CB_WL = NB_H * 2 * N   # additive-C cols per local window (3072)
CP_WL = 4 * 2 * N      # pre-scaled C' cols per local window (2048)
A16 = 128.0 / float(np.log(2.0))     # Schraudolph scale for bf16-via-int16
B16 = 127.0 * 128.0 - 5.09           # Schraudolph bias (round-to-nearest c)

_NC_CACHE = {}


def build_bass(trace_sim=False):
    nc = bacc.Bacc("TRN2", target_bir_lowering=False, debug=False, num_devices=M_CORES)
    # 4 heads per half-tile: partition 32*g + d, free (q|k, n) — bf16 keeps
    # FWL enabled so LDWEIGHTS (53ns) hides under the 107ns rhs stream, and
    # tile_position row groups let 4 heads' QK matmuls overlap on silicon.
    qk8 = nc.declare_dram_parameter("qk8", [NI, 128, 2 * 2 * N], mybir.dt.bfloat16, isOutput=False)
    vA = nc.declare_dram_parameter("vA", [NI, 128, 2 * H * 17], mybir.dt.bfloat16, isOutput=False)
    Cb = nc.declare_dram_parameter("Cb", [128, WL * CB_WL], mybir.dt.bfloat16, isOutput=False)
    Cp = nc.declare_dram_parameter("Cp", [128, WL * CP_WL], mybir.dt.float32, isOutput=False)
    Ib = nc.declare_dram_parameter("Ib", [128, 128], mybir.dt.bfloat16, isOutput=False)
    out = nc.declare_dram_parameter("out", [NI, N, HD], mybir.dt.float32, isOutput=True)

    FP32 = mybir.dt.float32
    BF = mybir.dt.bfloat16
    I16 = mybir.dt.int16
    Exp = mybir.ActivationFunctionType.Exp
    DR = mybir.MatmulPerfMode.DoubleRow

    with tile.TileContext(nc, trace_sim=trace_sim) as tc:
        with (
            tc.tile_pool(name="const", bufs=1) as constp,
            tc.tile_pool(name="qk", bufs=4) as qkp,
            tc.tile_pool(name="vp", bufs=4) as vp,
            tc.tile_pool(name="pp", bufs=6) as ppool,
            tc.tile_pool(name="op", bufs=3) as opool,
            tc.tile_pool(name="ps", bufs=3, space=bass.MemorySpace.PSUM) as psp,
            tc.tile_pool(name="av", bufs=2, space=bass.MemorySpace.PSUM) as avp,
        ):
            ctile = constp.tile([128, WL * CB_WL], BF)
            cptile = constp.tile([128, WL * CP_WL], FP32)
            itile = constp.tile([128, 128], BF)
            nc.gpsimd.dma_start(itile[:], Ib[:])

            def fetch_c(wl):
                # per-window C chunks, emitted just-in-time so window 0 isn't
                # queued behind the full 14 MiB of C on the Pool DMA FIFO
                nc.gpsimd.dma_start(ctile[:, wl * CB_WL:(wl + 1) * CB_WL], Cb[:, wl * CB_WL:(wl + 1) * CB_WL])
                nc.gpsimd.dma_start(cptile[:, wl * CP_WL:(wl + 1) * CP_WL], Cp[:, wl * CP_WL:(wl + 1) * CP_WL])

            fetch_c(0)
            fetch_c(1)
            for inst in range(NI):
                wl = inst // IMG
                if inst % IMG == 0 and wl + 2 < WL:
                    fetch_c(wl + 2)
                qktile = qkp.tile([128, 2 * 2 * N], BF)
                vtile = vp.tile([128, 2 * H * 17], BF)
                qk_eng = nc.sync if (inst % 4) != 3 else nc.gpsimd
                qk_eng.dma_start(qktile[:], qk8[inst])
                nc.gpsimd.dma_start(vtile[:], vA[inst])
                # partitions 32g+d hold heads h=4*half+g: free [half=2, qk=2, n=256]
                qk5 = qktile[:].rearrange("p (s q n) -> p s q n", s=2, q=2)

                avps = avp.tile([128, 2 * H * 17], FP32)
                for hp in (3, 0, 1, 2):  # DVE-path pair first per window
                    use_dve = (hp == 3) or (hp == 2 and inst % 8 < 5)
                    ps = psp.tile([128, 1024], FP32)
                    for hh in range(2):
                        h = 2 * hp + hh
                        hoff = hh * 512
                        if not use_dve:
                            # C[wl, h] first (start=True), QK accumulates on top
                            coff = wl * CB_WL + h * 2 * N
                            nc.tensor.matmul(
                                ps[:, hoff: hoff + 512],
                                itile[:],
                                ctile[:, coff: coff + 512],
                                start=True, stop=False,
                                skip_group_check=True,
                            )
                        half, g = h // 4, h % 4
                        qkh = qk5[32 * g: 32 * g + D, half]
                        for mc in range(2):
                            # S'[m,n]: lhsT = kT[d, m-chunk], rhs = qT[d, n]
                            nc.tensor.matmul(
                                ps[:, hoff + mc * 256: hoff + mc * 256 + 256],
                                qkh[:, 1, mc * 128:(mc + 1) * 128],
                                qkh[:, 0, :],
                                start=use_dve, stop=(mc == 1),
                                skip_group_check=True,
                                tile_position=(32 * g, 0),
                            )
                    if use_dve:
                        ptile = ppool.tile([128, 1024], I16, tag="pt")
                        poff = wl * CP_WL + (2 * hp - 4) * 512
                        nc.vector.scalar_tensor_tensor(
                            ptile[:], ps[:], A16, cptile[:, poff: poff + 1024],
                            mybir.AluOpType.mult, mybir.AluOpType.add,
                        )
                        pbf = ptile[:].bitcast(BF)
                    else:
                        ptile = ppool.tile([128, 1024], BF, tag="pt")
                        nc.scalar.activation(ptile[:], ps[:], Exp)
                        pbf = ptile[:]
                    # AV: out[n, d|denom] accumulated over m-chunks
                    for hh in range(2):
                        h = 2 * hp + hh
                        hoff = hh * 512
                        for nck in range(2):
                            for mc in range(2):
                                nc.tensor.matmul(
                                    avps[:, nck * (H * 17) + h * 17: nck * (H * 17) + h * 17 + 17],
                                    pbf[:, hoff + mc * 256 + nck * 128: hoff + mc * 256 + nck * 128 + 128],
                                    vtile[:, mc * (H * 17) + h * 17: mc * (H * 17) + h * 17 + 17],
                                    start=(mc == 0), stop=(mc == 1),
                                )

                otile = opool.tile([128, 2 * HD], FP32)
                rtile = opool.tile([128, 2 * H], FP32, tag="recip")
                av3 = avps[:].rearrange("p (nck h x) -> p nck h x", nck=2, h=H)
                nc.vector.reciprocal(
                    rtile[:].rearrange("p (nck h) -> p nck h", nck=2),
                    av3[:, :, :, 16],
                )
                nc.vector.tensor_mul(
                    otile[:].rearrange("p (nck h d) -> p nck h d", nck=2, h=H),
                    av3[:, :, :, 0:D],
                    rtile[:].rearrange("p (nck h) -> p nck h", nck=2)[:, :, :, None].broadcast_to([128, 2, H, D]),
                )
                # single out DMA: dram[(nck*128+p), hd] <- otile[p, (nck, hd)]
                nc.gpsimd.dma_start(
                    out[inst].rearrange("(nck p) hd -> p nck hd", nck=2),
                    otile[:].rearrange("p (nck hd) -> p nck hd", nck=2),
                )
    nc.compile()
    return nc


def _host_prep(q, k, v, table, index, mask):
    """Returns per-core input maps + the inverse b-index map."""
    # l2 normalize q, k (host): matches F.normalize(x, dim=-1)
    qn = q / np.maximum(np.sqrt((q * q).sum(-1, keepdims=True)), EPS)
    kn = k / np.maximum(np.sqrt((k * k).sum(-1, keepdims=True)), EPS)
    # 4-head row-group layout: [b, g, d(padded to 32), half, qk, n], h = 4*half+g
    qk8 = np.zeros((B_, 4, 32, 2, 2, N), np.float32)
    qk8[:, :, :D, :, 0] = qn.transpose(0, 1, 3, 2).reshape(B_, 2, 4, D, N).transpose(0, 2, 3, 1, 4)
    qk8[:, :, :D, :, 1] = kn.transpose(0, 1, 3, 2).reshape(B_, 2, 4, D, N).transpose(0, 2, 3, 1, 4)
    qk8 = qk8.reshape(B_, 128, 2 * 2 * N).astype(BF16)
    # v_aug [b, n, h, 17] -> [b, mc, 128, h, 17] -> [b, 128, mc*h*17]
    vA = np.empty((B_, N, H, 17), np.float32)
    vA[..., :16] = v.transpose(0, 2, 1, 3)
    vA[..., 16] = 1.0
    vA = vA.reshape(B_, 2, 128, H * 17).transpose(0, 2, 1, 3).reshape(B_, 128, 2 * H * 17).astype(BF16)
    # bias'[h, m, n] = table[index[n*256+m], h]
    bias = table[index.astype(np.int64)].reshape(N, N, H).transpose(2, 1, 0)  # [h, m, n]
    maskT = mask.transpose(0, 2, 1)  # [w, m, n]

    in_maps = []
    b_order = []
    ident = np.eye(128, dtype=BF16)
    for c in range(M_CORES):
        # device instance i <-> wl = i // IMG, img = i % IMG
        bs = np.array([img * NW + (c + M_CORES * wl) for wl in range(WL) for img in range(IMG)])
        b_order.append(bs)
        # C[wl, h, m, n] = bias'[h] + maskT[c + 8*wl]
        C = (bias[None, :, :, :] + maskT[c::M_CORES][:, None, :, :]).astype(np.float32)
        C = C.reshape(WL, H, 2, 128, N)
        # additive path: heads 0..5, bf16, [p, (wl h mc n)]
        Cb_ = C[:, :NB_H].transpose(3, 0, 1, 2, 4).reshape(128, WL * CB_WL).astype(BF16)
        # Schraudolph path: heads 4..7, fp32 pre-scaled A*C + B
        Cp_ = (A16 * C[:, 4:] + B16).transpose(3, 0, 1, 2, 4).reshape(128, WL * CP_WL).astype(np.float32)
        in_maps.append({
            "qk8": np.ascontiguousarray(qk8[bs]),
            "vA": np.ascontiguousarray(vA[bs]),
            "Cb": Cb_,
            "Cp": Cp_,
            "Ib": ident,
        })
    return in_maps, b_order


def kernel(q, k, v, table, index, mask):
    q = np.asarray(q, np.float32)
    k = np.asarray(k, np.float32)
    v = np.asarray(v, np.float32)
    table = np.asarray(table, np.float32)
    index = np.asarray(index)
    mask = np.asarray(mask, np.float32)

    in_maps, b_order = _host_prep(q, k, v, table, index, mask)

    if "nc" not in _NC_CACHE:
        _NC_CACHE["nc"] = build_bass()
    nc = _NC_CACHE["nc"]

    res = run_bass_kernel_spmd(nc, in_maps, core_ids=list(range(M_CORES)))
    out = np.empty((B_, N, HD), np.float32)
    for c in range(M_CORES):
        out[b_order[c]] = res.results[c]["out"]
    return out


if __name__ == "__main__":
    rng = np.random.default_rng(0)
    q = rng.standard_normal((B_, H, N, D), dtype=np.float32)
    k = rng.standard_normal((B_, H, N, D), dtype=np.float32)
    v = rng.standard_normal((B_, H, N, D), dtype=np.float32)
    table = rng.standard_normal((961, H), dtype=np.float32)
    index = rng.integers(0, 961, size=(N * N,)).astype(np.int64)
    mask = rng.standard_normal((NW, N, N), dtype=np.float32)
    o = kernel(q=q, k=k, v=v, table=table, index=index, mask=mask)
    print("out", o.shape, o.dtype, float(np.abs(o).mean()))



# revision 4
# speedup vs baseline: 1.0674x; 1.0674x over previous
"""Trainium2 Bass kernel for Swin-style windowed cosine attention.

Problem: nn_Attention_8100308321041
  q,k,v: [512, 8, 256, 16] f32; table: [961, 8]; index: [65536] i64;
  mask: [64, 256, 256] f32; out: [512, 256, 128] f32.

Strategy (8 NeuronCores, pure data-parallel):
  - Shard window-instances b by (b % 64) % 8 == core  -> 64 instances/core,
    ordered (wl, img) so each per-window bias+mask chunk is fetched once and
    reused across 8 images while the next chunk prefetches.
  - Host prep: l2-normalize q/k, transpose to a bf16 4-head row-group layout
    (partition 32*g + d) so four heads' K=16 QK matmuls occupy distinct PE
    row groups; gather table[index] -> bias, combine bias+mask into C (bf16)
    and C' = A*C + B (fp32, Schraudolph path); v_aug with a ones column
    (fused softmax denominator).
  - Device per instance (restructured v2):
      * identity-preload C for ACT-path pairs (one [128,1024] matmul each)
      * QK burst: 16 K=16 matmuls in 4-way row-group concurrency
        (no full-width matmuls interleaved -> ~4x matmul overlap)
      * exp: ACT pairs on ScalarE (exp bf16), STT pairs on VectorE
        (Schraudolph int16 bitcast, C'-add fused)
      * AV + epilogue for the PREVIOUS instance are emitted after this
        instance's QK so the PE never head-of-line blocks on exp
      * out stored bf16 (host converts to fp32)
"""

import os
import sys

sys.path.insert(0, "/opt/trn_rl_repo")

import numpy as np
import ml_dtypes

import concourse.bass as bass
import concourse.bacc as bacc
import concourse.mybir as mybir
from concourse import tile
from concourse.bass_utils import run_bass_kernel_spmd

BF16 = ml_dtypes.bfloat16

B_, H, N, D = 512, 8, 256, 16
NW = 64          # windows per image
M_CORES = 8
IMG = B_ // NW   # 8 images
WL = NW // M_CORES  # 8 distinct windows per core
NI = IMG * WL    # 64 instances per core
HD = H * D       # 128
EPS = 1e-12
NB_H = 6         # heads with additive-C (bf16) available (ACT path)
CB_WL = NB_H * 2 * N   # additive-C cols per local window (3072)
CP_WL = 4 * 2 * N      # pre-scaled C' cols per local window (2048)
A16 = 128.0 / float(np.log(2.0))     # Schraudolph scale for bf16-via-int16
B16 = 127.0 * 128.0 - 5.09           # Schraudolph bias (round-to-nearest c)

_NC_CACHE = {}


def build_bass(trace_sim=False):
    nc = bacc.Bacc("TRN2", target_bir_lowering=False, debug=False, num_devices=M_CORES)
    qk8 = nc.declare_dram_parameter("qk8", [NI, 128, 2 * 2 * N], mybir.dt.bfloat16, isOutput=False)
    vA = nc.declare_dram_parameter("vA", [NI, 128, 2 * H * 17], mybir.dt.bfloat16, isOutput=False)
    Cb = nc.declare_dram_parameter("Cb", [128, WL * CB_WL], mybir.dt.bfloat16, isOutput=False)
    Cp = nc.declare_dram_parameter("Cp", [128, WL * CP_WL], mybir.dt.float32, isOutput=False)
    Ib = nc.declare_dram_parameter("Ib", [128, 128], mybir.dt.bfloat16, isOutput=False)
    out = nc.declare_dram_parameter("out", [NI, N, HD], mybir.dt.float32, isOutput=True)

    FP32 = mybir.dt.float32
    BF = mybir.dt.bfloat16
    I16 = mybir.dt.int16
    Exp = mybir.ActivationFunctionType.Exp

    with tile.TileContext(nc, trace_sim=trace_sim) as tc:
        with (
            tc.tile_pool(name="const", bufs=1) as constp,
            tc.tile_pool(name="qk", bufs=4) as qkp,
            tc.tile_pool(name="vp", bufs=4) as vp,
            tc.tile_pool(name="pp", bufs=10) as ppool,
            tc.tile_pool(name="op", bufs=3) as opool,
            tc.tile_pool(name="ps", bufs=3, space=bass.MemorySpace.PSUM) as psp,
            tc.tile_pool(name="av", bufs=2, space=bass.MemorySpace.PSUM) as avp,
        ):
            ctile = constp.tile([128, WL * CB_WL], BF)
            cptile = constp.tile([128, WL * CP_WL], FP32)
            itile = constp.tile([128, 128], BF)
            nc.gpsimd.dma_start(itile[:], Ib[:])

            def fetch_c(wl):
                nc.gpsimd.dma_start(ctile[:, wl * CB_WL:(wl + 1) * CB_WL], Cb[:, wl * CB_WL:(wl + 1) * CB_WL])
                nc.gpsimd.dma_start(cptile[:, wl * CP_WL:(wl + 1) * CP_WL], Cp[:, wl * CP_WL:(wl + 1) * CP_WL])

            fetch_c(0)
            fetch_c(1)

            pending = []  # deferred AV+epilogue work from the previous instance

            def flush_pending():
                # AV + epilogue + out-DMA for the previous instance
                for (p_inst, p_vtile, p_pb) in pending:
                    avps = avp.tile([128, 2 * H * 17], FP32, name="avps")
                    for pr in range(4):
                        pbf = p_pb[pr]
                        for hh in range(2):
                            h = 2 * pr + hh
                            hoff = hh * 512
                            for nck in range(2):
                                for mc in range(2):
                                    nc.tensor.matmul(
                                        avps[:, nck * (H * 17) + h * 17: nck * (H * 17) + h * 17 + 17],
                                        pbf[:, hoff + mc * 256 + nck * 128: hoff + mc * 256 + nck * 128 + 128],
                                        p_vtile[:, mc * (H * 17) + h * 17: mc * (H * 17) + h * 17 + 17],
                                        start=(mc == 0), stop=(mc == 1),
                                    )
                    otile = opool.tile([128, 2 * HD], FP32, name="otile")
                    rtile = opool.tile([128, 2 * H], FP32, name="rtile")
                    av3 = avps[:].rearrange("p (nck h x) -> p nck h x", nck=2, h=H)
                    nc.vector.reciprocal(
                        rtile[:].rearrange("p (nck h) -> p nck h", nck=2),
                        av3[:, :, :, 16],
                    )
                    nc.vector.tensor_mul(
                        otile[:].rearrange("p (nck h d) -> p nck h d", nck=2, h=H),
                        av3[:, :, :, 0:D],
                        rtile[:].rearrange("p (nck h) -> p nck h", nck=2)[:, :, :, None].broadcast_to([128, 2, H, D]),
                    )
                    nc.gpsimd.dma_start(
                        out[p_inst].rearrange("(nck p) hd -> p nck hd", nck=2),
                        otile[:].rearrange("p (nck hd) -> p nck hd", nck=2),
                    )
                pending.clear()

            for inst in range(NI):
                wl = inst // IMG
                if inst % IMG == 0 and wl + 2 < WL:
                    fetch_c(wl + 2)
                qktile = qkp.tile([128, 2 * 2 * N], BF, name="qktile")
                vtile = vp.tile([128, 2 * H * 17], BF, name="vtile")
                qk_eng = nc.sync if (inst % 4) != 3 else nc.gpsimd
                qk_eng.dma_start(qktile[:], qk8[inst])
                nc.gpsimd.dma_start(vtile[:], vA[inst])
                qk5 = qktile[:].rearrange("p (s q n) -> p s q n", s=2, q=2)

                # pair paths: pairs 0,1 always ACT; pair2 alternates; pair3 STT
                is_act = [True, True, (inst % 2 == 0), False]

                pstiles = []
                for pr in range(4):
                    ps = psp.tile([128, 1024], FP32, name="ps")
                    pstiles.append(ps)

                # Phase 1: identity C-preloads for ACT pairs (one MM per pair)
                for pr in range(4):
                    if is_act[pr]:
                        coff = wl * CB_WL + (2 * pr) * 2 * N
                        for hh in range(2):
                            nc.tensor.matmul(
                                pstiles[pr][:, hh * 512:(hh + 1) * 512],
                                itile[:], ctile[:, coff + hh * 512: coff + (hh + 1) * 512],
                                start=True, stop=False, skip_group_check=True,
                            )

                # Phase 2: QK bursts (row-group concurrent, no full-width MMs)
                for half in range(2):
                    for mc in range(2):
                        for g in range(4):
                            h = 4 * half + g
                            pr = h // 2
                            hoff = (h % 2) * 512
                            qkh = qk5[32 * g: 32 * g + D, half]
                            nc.tensor.matmul(
                                pstiles[pr][:, hoff + mc * 256: hoff + mc * 256 + 256],
                                qkh[:, 1, mc * 128:(mc + 1) * 128],
                                qkh[:, 0, :],
                                start=(not is_act[pr]) and mc == 0,
                                stop=(mc == 1),
                                skip_group_check=True,
                                tile_position=(32 * g, 0),
                            )

                # Phase 3: exp per pair
                ptiles = []
                for pr in range(4):
                    if is_act[pr]:
                        ptile = ppool.tile([128, 1024], BF, name="pt", tag="pt")
                        nc.scalar.activation(ptile[:], pstiles[pr][:], Exp)
                        ptiles.append(ptile[:])
                    else:
                        ptile = ppool.tile([128, 1024], I16, name="pt", tag="pt")
                        poff = wl * CP_WL + (2 * pr - 4) * 512
                        nc.vector.scalar_tensor_tensor(
                            ptile[:], pstiles[pr][:], A16, cptile[:, poff: poff + 1024],
                            mybir.AluOpType.mult, mybir.AluOpType.add,
                        )
                        ptiles.append(ptile[:].bitcast(BF))

                # Phase 4: AV + epilogue (same instance for bisection)
                pending.append((inst, vtile, ptiles))
                flush_pending()

            flush_pending()
    nc.compile()
    return nc


def _host_prep(q, k, v, table, index, mask):
    """Returns per-core input maps + the inverse b-index map."""
    qn = q / np.maximum(np.sqrt((q * q).sum(-1, keepdims=True)), EPS)
    kn = k / np.maximum(np.sqrt((k * k).sum(-1, keepdims=True)), EPS)
    # 4-head row-group layout: [b, g, d(padded to 32), half, qk, n], h = 4*half+g
    qk8 = np.zeros((B_, 4, 32, 2, 2, N), np.float32)
    qk8[:, :, :D, :, 0] = qn.transpose(0, 1, 3, 2).reshape(B_, 2, 4, D, N).transpose(0, 2, 3, 1, 4)
    qk8[:, :, :D, :, 1] = kn.transpose(0, 1, 3, 2).reshape(B_, 2, 4, D, N).transpose(0, 2, 3, 1, 4)
    qk8 = qk8.reshape(B_, 128, 2 * 2 * N).astype(BF16)
    # v_aug [b, n, h, 17] -> [b, mc, 128, h, 17] -> [b, 128, mc*h*17]
    vA = np.empty((B_, N, H, 17), np.float32)
    vA[..., :16] = v.transpose(0, 2, 1, 3)
    vA[..., 16] = 1.0
    vA = vA.reshape(B_, 2, 128, H * 17).transpose(0, 2, 1, 3).reshape(B_, 128, 2 * H * 17).astype(BF16)
    # bias'[h, m, n] = table[index[n*256+m], h]
    bias = table[index.astype(np.int64)].reshape(N, N, H).transpose(2, 1, 0)  # [h, m, n]
    maskT = mask.transpose(0, 2, 1)  # [w, m, n]

    in_maps = []
    b_order = []
    ident = np.eye(128, dtype=BF16)
    for c in range(M_CORES):
        bs = np.array([img * NW + (c + M_CORES * wl) for wl in range(WL) for img in range(IMG)])
        b_order.append(bs)
        C = (bias[None, :, :, :] + maskT[c::M_CORES][:, None, :, :]).astype(np.float32)
        C = C.reshape(WL, H, 2, 128, N)
        Cb_ = C[:, :NB_H].transpose(3, 0, 1, 2, 4).reshape(128, WL * CB_WL).astype(BF16)
        Cp_ = (A16 * C[:, 4:] + B16).transpose(3, 0, 1, 2, 4).reshape(128, WL * CP_WL).astype(np.float32)
        in_maps.append({
            "qk8": np.ascontiguousarray(qk8[bs]),
            "vA": np.ascontiguousarray(vA[bs]),
            "Cb": Cb_,
            "Cp": Cp_,
            "Ib": ident,
        })
    return in_maps, b_order


def kernel(q, k, v, table, index, mask):
    q = np.asarray(q, np.float32)
    k = np.asarray(k, np.float32)
    v = np.asarray(v, np.float32)
    table = np.asarray(table, np.float32)
    index = np.asarray(index)
    mask = np.asarray(mask, np.float32)

    in_maps, b_order = _host_prep(q, k, v, table, index, mask)

    if "nc" not in _NC_CACHE:
        _NC_CACHE["nc"] = build_bass()
    nc = _NC_CACHE["nc"]

    res = run_bass_kernel_spmd(nc, in_maps, core_ids=list(range(M_CORES)))
    out = np.empty((B_, N, HD), np.float32)
    for c in range(M_CORES):
        out[b_order[c]] = res.results[c]["out"]
    return out


if __name__ == "__main__":
    rng = np.random.default_rng(0)
    q = rng.standard_normal((B_, H, N, D), dtype=np.float32)
    k = rng.standard_normal((B_, H, N, D), dtype=np.float32)
    v = rng.standard_normal((B_, H, N, D), dtype=np.float32)
    table = rng.standard_normal((961, H), dtype=np.float32)
    index = rng.integers(0, 961, size=(N * N,)).astype(np.int64)
    mask = rng.standard_normal((NW, N, N), dtype=np.float32)
    o = kernel(q=q, k=k, v=v, table=table, index=index, mask=mask)
    print("out", o.shape, o.dtype, float(np.abs(o).mean()))


# revision 11
# speedup vs baseline: 1.2654x; 1.1855x over previous
"""Trainium2 Bass kernel for Swin-style windowed cosine attention.

Problem: nn_Attention_8100308321041
  q,k,v: [512, 8, 256, 16] f32; table: [961, 8]; index: [65536] i64;
  mask: [64, 256, 256] f32; out: [512, 256, 128] f32.

Strategy (8 NeuronCores, pure data-parallel):
  - Shard window-instances b by (b % 64) % 8 == core  -> 64 instances/core,
    ordered (wl, img) so each per-window bias+mask chunk is fetched once and
    reused across 8 images while the next chunk prefetches.
  - Host prep: l2-normalize q/k, transpose to a bf16 4-head row-group layout
    (partition 32*g + d) so four heads' K=16 QK matmuls occupy distinct PE
    row groups; gather table[index] -> bias, combine bias+mask into C (bf16)
    and C' = A*C + B (fp32, Schraudolph path); v_aug with a ones column
    (fused softmax denominator).
  - Device per instance (restructured v2):
      * identity-preload C for ACT-path pairs (one [128,1024] matmul each)
      * QK burst: 16 K=16 matmuls in 4-way row-group concurrency
        (no full-width matmuls interleaved -> ~4x matmul overlap)
      * exp: ACT pairs on ScalarE (exp bf16), STT pairs on VectorE
        (Schraudolph int16 bitcast, C'-add fused)
      * AV + epilogue for the PREVIOUS instance are emitted after this
        instance's QK so the PE never head-of-line blocks on exp
      * out stored bf16 (host converts to fp32)
"""

import os
import sys

sys.path.insert(0, "/opt/trn_rl_repo")

import numpy as np
import ml_dtypes

import concourse.bass as bass
import concourse.bacc as bacc
import concourse.mybir as mybir
from concourse import tile
from concourse.bass_utils import run_bass_kernel_spmd

BF16 = ml_dtypes.bfloat16

B_, H, N, D = 512, 8, 256, 16
NW = 64          # windows per image
M_CORES = 8
IMG = B_ // NW   # 8 images
WL = NW // M_CORES  # 8 distinct windows per core
NI = IMG * WL    # 64 instances per core
HD = H * D       # 128
EPS = 1e-12
NB_H = 6         # heads with additive-C (bf16) available (ACT path)
CB_WL = NB_H * 2 * N   # additive-C cols per local window (3072)
CP_WL = 4 * 2 * N      # pre-scaled C' cols per local window (2048)
A16 = 128.0 / float(np.log(2.0))     # Schraudolph scale for bf16-via-int16
B16 = 127.0 * 128.0 - 5.09           # Schraudolph bias (round-to-nearest c)

_NC_CACHE = {}


def build_bass(trace_sim=False):
    nc = bacc.Bacc("TRN2", target_bir_lowering=False, debug=False, num_devices=M_CORES)
    qk8 = nc.declare_dram_parameter("qk8", [NI, 128, 2 * 2 * N], mybir.dt.bfloat16, isOutput=False)
    vA = nc.declare_dram_parameter("vA", [NI, 128, 2 * H * 17], mybir.dt.bfloat16, isOutput=False)
    Cb = nc.declare_dram_parameter("Cb", [128, WL * CB_WL], mybir.dt.bfloat16, isOutput=False)
    Cp = nc.declare_dram_parameter("Cp", [128, WL * CP_WL], mybir.dt.float32, isOutput=False)
    Ib = nc.declare_dram_parameter("Ib", [128, 128], mybir.dt.bfloat16, isOutput=False)
    out = nc.declare_dram_parameter("out", [NI, N, HD], mybir.dt.float32, isOutput=True)

    FP32 = mybir.dt.float32
    BF = mybir.dt.bfloat16
    I16 = mybir.dt.int16
    Exp = mybir.ActivationFunctionType.Exp

    with tile.TileContext(nc, trace_sim=trace_sim) as tc:
        with (
            tc.tile_pool(name="const", bufs=1) as constp,
            tc.tile_pool(name="qk", bufs=4) as qkp,
            tc.tile_pool(name="vp", bufs=4) as vp,
            tc.tile_pool(name="pp", bufs=10) as ppool,
            tc.tile_pool(name="op", bufs=3) as opool,
            tc.tile_pool(name="ps", bufs=4, space=bass.MemorySpace.PSUM) as psp,
        ):
            ctile = constp.tile([128, WL * CB_WL], BF)
            cptile = constp.tile([128, WL * CP_WL], FP32)
            itile = constp.tile([128, 128], BF)
            nc.gpsimd.dma_start(itile[:], Ib[:])

            def fetch_c(wl):
                nc.gpsimd.dma_start(ctile[:, wl * CB_WL:(wl + 1) * CB_WL], Cb[:, wl * CB_WL:(wl + 1) * CB_WL])
                nc.gpsimd.dma_start(cptile[:, wl * CP_WL:(wl + 1) * CP_WL], Cp[:, wl * CP_WL:(wl + 1) * CP_WL])

            fetch_c(0)
            fetch_c(1)

            pending = []  # deferred AV+epilogue work from the previous instance

            def flush_pending():
                # AV + epilogue + out-DMA for the previous instance
                for (p_inst, p_vtile, p_pb) in pending:
                    # avps rides the psum pool's recycle slot: it only becomes
                    # needed once exp(pair0) freed a buffer anyway
                    avps_full = psp.tile([128, 1024], FP32, name="avps", tag="ps")
                    avps = avps_full[:, 0:2 * H * 17]
                    for pr in range(4):
                        pbf = p_pb[pr]
                        for hh in range(2):
                            h = 2 * pr + hh
                            hoff = hh * 512
                            for nck in range(2):
                                for mc in range(2):
                                    nc.tensor.matmul(
                                        avps[:, nck * (H * 17) + h * 17: nck * (H * 17) + h * 17 + 17],
                                        pbf[:, hoff + mc * 256 + nck * 128: hoff + mc * 256 + nck * 128 + 128],
                                        p_vtile[:, mc * (H * 17) + h * 17: mc * (H * 17) + h * 17 + 17],
                                        start=(mc == 0), stop=(mc == 1),
                                    )
                    otile = opool.tile([128, 2 * HD], FP32, name="otile")
                    rtile = opool.tile([128, 2 * H], FP32, name="rtile")
                    av3 = avps.rearrange("p (nck h x) -> p nck h x", nck=2, h=H)
                    nc.vector.reciprocal(
                        rtile[:].rearrange("p (nck h) -> p nck h", nck=2),
                        av3[:, :, :, 16],
                    )
                    nc.vector.tensor_mul(
                        otile[:].rearrange("p (nck h d) -> p nck h d", nck=2, h=H),
                        av3[:, :, :, 0:D],
                        rtile[:].rearrange("p (nck h) -> p nck h", nck=2)[:, :, :, None].broadcast_to([128, 2, H, D]),
                    )
                    nc.gpsimd.dma_start(
                        out[p_inst].rearrange("(nck p) hd -> p nck hd", nck=2),
                        otile[:].rearrange("p (nck hd) -> p nck hd", nck=2),
                    )
                pending.clear()

            def fetch_inst(i):
                qt = qkp.tile([128, 2 * 2 * N], BF, name="qktile")
                vt = vp.tile([128, 2 * H * 17], BF, name="vtile")
                qk_eng = nc.sync if (i % 4) != 3 else nc.gpsimd
                qk_eng.dma_start(qt[:], qk8[i])
                nc.gpsimd.dma_start(vt[:], vA[i])
                return qt, vt

            inst_tiles = {0: fetch_inst(0)}

            for inst in range(NI):
                wl = inst // IMG
                if inst % IMG == 0 and wl + 2 < WL:
                    fetch_c(wl + 2)
                qktile, vtile = inst_tiles.pop(inst)
                if inst + 1 < NI:
                    inst_tiles[inst + 1] = fetch_inst(inst + 1)
                qk5 = qktile[:].rearrange("p (s q n) -> p s q n", s=2, q=2)

                # AV + epilogue of the PREVIOUS instance go first: its avps
                # allocation naturally waits for exp(pair0) of the previous
                # instance, freeing the PE to chew on it while this
                # instance's DMAs land
                flush_pending()

                # pair paths: pairs 0,1 always ACT; pair2 alternates; pair3 STT
                is_act = [True, True, (inst % 2 == 0), False]

                pstiles = []
                for pr in range(4):
                    ps = psp.tile([128, 1024], FP32, name="ps", tag="ps")
                    pstiles.append(ps)

                # Phase 1: identity C-preloads for ACT pairs (one MM per pair)
                for pr in range(4):
                    if is_act[pr]:
                        coff = wl * CB_WL + (2 * pr) * 2 * N
                        for hh in range(2):
                            nc.tensor.matmul(
                                pstiles[pr][:, hh * 512:(hh + 1) * 512],
                                itile[:], ctile[:, coff + hh * 512: coff + (hh + 1) * 512],
                                start=True, stop=False, skip_group_check=True,
                            )

                # Phase 2: QK bursts (row-group concurrent, no full-width MMs)
                for half in range(2):
                    for mc in range(2):
                        for g in range(4):
                            h = 4 * half + g
                            pr = h // 2
                            hoff = (h % 2) * 512
                            qkh = qk5[32 * g: 32 * g + D, half]
                            nc.tensor.matmul(
                                pstiles[pr][:, hoff + mc * 256: hoff + mc * 256 + 256],
                                qkh[:, 1, mc * 128:(mc + 1) * 128],
                                qkh[:, 0, :],
                                start=(not is_act[pr]) and mc == 0,
                                stop=(mc == 1),
                                skip_group_check=True,
                                tile_position=(32 * g, 0),
                            )

                # Phase 3: exp per pair
                ptiles = []
                for pr in range(4):
                    if is_act[pr]:
                        ptile = ppool.tile([128, 1024], BF, name="pt", tag="pt")
                        nc.scalar.activation(ptile[:], pstiles[pr][:], Exp)
                        ptiles.append(ptile[:])
                    else:
                        ptile = ppool.tile([128, 1024], I16, name="pt", tag="pt")
                        poff = wl * CP_WL + (2 * pr - 4) * 512
                        nc.vector.scalar_tensor_tensor(
                            ptile[:], pstiles[pr][:], A16, cptile[:, poff: poff + 1024],
                            mybir.AluOpType.mult, mybir.AluOpType.add,
                        )
                        ptiles.append(ptile[:].bitcast(BF))

                pending.append((inst, vtile, ptiles))

            flush_pending()
    nc.compile()
    return nc


def _host_prep(q, k, v, table, index, mask):
    """Returns per-core input maps + the inverse b-index map."""
    qn = q / np.maximum(np.sqrt((q * q).sum(-1, keepdims=True)), EPS)
    kn = k / np.maximum(np.sqrt((k * k).sum(-1, keepdims=True)), EPS)
    # 4-head row-group layout: [b, g, d(padded to 32), half, qk, n], h = 4*half+g
    qk8 = np.zeros((B_, 4, 32, 2, 2, N), np.float32)
    qk8[:, :, :D, :, 0] = qn.transpose(0, 1, 3, 2).reshape(B_, 2, 4, D, N).transpose(0, 2, 3, 1, 4)
    qk8[:, :, :D, :, 1] = kn.transpose(0, 1, 3, 2).reshape(B_, 2, 4, D, N).transpose(0, 2, 3, 1, 4)
    qk8 = qk8.reshape(B_, 128, 2 * 2 * N).astype(BF16)
    # v_aug [b, n, h, 17] -> [b, mc, 128, h, 17] -> [b, 128, mc*h*17]
    vA = np.empty((B_, N, H, 17), np.float32)
    vA[..., :16] = v.transpose(0, 2, 1, 3)
    vA[..., 16] = 1.0
    vA = vA.reshape(B_, 2, 128, H * 17).transpose(0, 2, 1, 3).reshape(B_, 128, 2 * H * 17).astype(BF16)
    # bias'[h, m, n] = table[index[n*256+m], h]
    bias = table[index.astype(np.int64)].reshape(N, N, H).transpose(2, 1, 0)  # [h, m, n]
    maskT = mask.transpose(0, 2, 1)  # [w, m, n]

    in_maps = []
    b_order = []
    ident = np.eye(128, dtype=BF16)
    for c in range(M_CORES):
        bs = np.array([img * NW + (c + M_CORES * wl) for wl in range(WL) for img in range(IMG)])
        b_order.append(bs)
        C = (bias[None, :, :, :] + maskT[c::M_CORES][:, None, :, :]).astype(np.float32)
        C = C.reshape(WL, H, 2, 128, N)
        Cb_ = C[:, :NB_H].transpose(3, 0, 1, 2, 4).reshape(128, WL * CB_WL).astype(BF16)
        Cp_ = (A16 * C[:, 4:] + B16).transpose(3, 0, 1, 2, 4).reshape(128, WL * CP_WL).astype(np.float32)
        in_maps.append({
            "qk8": np.ascontiguousarray(qk8[bs]),
            "vA": np.ascontiguousarray(vA[bs]),
            "Cb": Cb_,
            "Cp": Cp_,
            "Ib": ident,
        })
    return in_maps, b_order


def kernel(q, k, v, table, index, mask):
    q = np.asarray(q, np.float32)
    k = np.asarray(k, np.float32)
    v = np.asarray(v, np.float32)
    table = np.asarray(table, np.float32)
    index = np.asarray(index)
    mask = np.asarray(mask, np.float32)

    in_maps, b_order = _host_prep(q, k, v, table, index, mask)

    if "nc" not in _NC_CACHE:
        _NC_CACHE["nc"] = build_bass()
    nc = _NC_CACHE["nc"]

    res = run_bass_kernel_spmd(nc, in_maps, core_ids=list(range(M_CORES)))
    out = np.empty((B_, N, HD), np.float32)
    for c in range(M_CORES):
        out[b_order[c]] = res.results[c]["out"]
    return out


if __name__ == "__main__":
    rng = np.random.default_rng(0)
    q = rng.standard_normal((B_, H, N, D), dtype=np.float32)
    k = rng.standard_normal((B_, H, N, D), dtype=np.float32)
    v = rng.standard_normal((B_, H, N, D), dtype=np.float32)
    table = rng.standard_normal((961, H), dtype=np.float32)
    index = rng.integers(0, 961, size=(N * N,)).astype(np.int64)
    mask = rng.standard_normal((NW, N, N), dtype=np.float32)
    o = kernel(q=q, k=k, v=v, table=table, index=index, mask=mask)
    print("out", o.shape, o.dtype, float(np.abs(o).mean()))
